# revision 56
# baseline (speedup 1.0000x reference)
"""GQA self-attention kernel for Trainium2, sharded over 8 NeuronCores.

Problem: x[4, 2048, 1024], 16 heads / 4 KV groups / head_dim 64.
Sharding: batch (4) x head-half (2 KV groups each) = 8 cores.

Per-core dataflow (all "transposed world": features on partitions):
  xT[1024,2048] -> qT[512,2048], kT[128,2048], vT[128,2048]
      (PE matmuls; x and q/k/v weights stream in bf16, accumulate fp32)
  vT --PE transpose--> v_aug[seq,65] tiles (ones column for softmax sums)
  scores s[k,q] = kT_g^T(d,kpos) . qT_h(d,q)  (K=64, f32r; the two heads of a
      pair go to opposite PE-array row halves so hardware overlaps them)
  p = exp(s/8)  (ACT engine, 1024-wide tiles: both heads of a pair per kt)
  av[65,q] += v_aug^T p  (row 64 = softmax denominator)
  avT_norm = av[0:64] * recip(av[64])  (recips on DVE; one selector matmul
      broadcasts both heads' 1/denom rows; applied via DVE multiply)
  yT[e,q] = Wo_p^T . avT_norm  -> DRAM
Host: y[b] = (yT[2b] + yT[2b+1]).T + bo

Schedule: kv projections for all chunks first, then attention streams per
q-chunk. The next chunk's q-projection and the previous chunk's
out-projection matmuls are pumped one-per-kt into the ACT-bound attention
loop; each pair's normalization is deferred into the next pair's first
iterations so the scores pipeline never stalls.
"""

import sys
import numpy as np
import ml_dtypes

sys.path.insert(0, "/opt/trn_rl_repo")

from contextlib import ExitStack

import concourse.bass as bass
import concourse.bacc as bacc
import concourse.mybir as mybir
from concourse import tile
from concourse.bass_utils import run_bass_kernel_spmd

F32 = mybir.dt.float32
F32R = mybir.dt.float32r
BF16 = mybir.dt.bfloat16

B, S, E = 4, 2048, 1024
NUM_HEADS, NUM_GROUPS, D = 16, 4, 64
CQ = 512          # q cols per core (8 heads)
CK = 128          # kv cols per core (2 groups)
ET = E // 128     # 8 embed K-tiles
SC = S // 512     # 4 seq chunks of 512
KT = S // 128     # 16 key tiles of 128
QT = CQ // 128    # 4 qT partition tiles
SCALE = 1.0 / np.sqrt(np.float32(D))

_NC_CACHE = {}


def build_nc():
    nc = bacc.Bacc(None, target_bir_lowering=False)

    # partition-major layouts: [128, et, ...] so one DMA covers several
    # 128-row blocks; x and qkv weights are bf16 to halve the critical DMA
    xTd = nc.dram_tensor("xTd", [128, ET, S], BF16, kind="ExternalInput")
    wqd = nc.dram_tensor("wqd", [128, ET, CQ], BF16, kind="ExternalInput")
    wkd = nc.dram_tensor("wkd", [128, ET, CK], BF16, kind="ExternalInput")
    wvd = nc.dram_tensor("wvd", [128, ET, CK], BF16, kind="ExternalInput")
    wod = nc.dram_tensor("wod", [128, QT, E], F32R, kind="ExternalInput")
    bqd = nc.dram_tensor("bqd", [128, QT], F32, kind="ExternalInput")
    bkd = nc.dram_tensor("bkd", [128, 1], F32, kind="ExternalInput")
    bvd = nc.dram_tensor("bvd", [128, 1], F32, kind="ExternalInput")
    identd = nc.dram_tensor("identd", [128, 128], F32R, kind="ExternalInput")
    onesd = nc.dram_tensor("onesd", [128, 64], F32R, kind="ExternalInput")
    yT = nc.dram_tensor("yT", [E, S], F32, kind="ExternalOutput")

    ADD = mybir.AluOpType.add
    MUL = mybir.AluOpType.mult
    EXP = mybir.ActivationFunctionType.Exp

    with tile.TileContext(nc) as tc, ExitStack() as ctx, \
            nc.allow_low_precision(reason="attention math in f32r; x/qkv "
                                   "weight streams bf16 within tolerance"):
        const = ctx.enter_context(tc.tile_pool(name="const", bufs=1))
        wpool = ctx.enter_context(tc.tile_pool(name="wpool", bufs=1))
        big = ctx.enter_context(tc.tile_pool(name="big", bufs=1))
        x0pool = ctx.enter_context(tc.tile_pool(name="x0pool", bufs=1))
        xpool = ctx.enter_context(tc.tile_pool(name="xpool", bufs=3))
        ppool = ctx.enter_context(tc.tile_pool(name="ppool", bufs=4))
        avpool = ctx.enter_context(tc.tile_pool(name="avpool", bufs=3))
        ypool = ctx.enter_context(tc.tile_pool(name="ypool", bufs=2))
        npool = ctx.enter_context(tc.tile_pool(name="npool", bufs=4))
        # PSUM: psA 2x[128,1024] (4 banks) + psAV 2x[128,512] (2) + psY 2x (2)
        psA = ctx.enter_context(tc.tile_pool(name="psA", bufs=2, space="PSUM"))
        psAV = ctx.enter_context(tc.tile_pool(name="psAV", bufs=2, space="PSUM"))
        psY = ctx.enter_context(tc.tile_pool(name="psY", bufs=2, space="PSUM"))

        # ---- x chunk 0 + kv weights: the prologue-critical DMAs go first ----
        # 8-way split so 8 DMA engines fetch the first chunk concurrently
        xt0 = x0pool.tile([128, ET, 512], BF16, name="xt0")
        for et in range(ET):
            nc.sync.dma_start(out=xt0[:, et, :], in_=xTd[:, et, 0:512])
        wk_sb = wpool.tile([128, ET, CK], BF16)
        nc.sync.dma_start(out=wk_sb[:, 0:4, :], in_=wkd[:, 0:4, :])
        nc.sync.dma_start(out=wk_sb[:, 4:8, :], in_=wkd[:, 4:8, :])
        wv_sb = wpool.tile([128, ET, CK], BF16)
        nc.sync.dma_start(out=wv_sb[:, 0:4, :], in_=wvd[:, 0:4, :])
        nc.sync.dma_start(out=wv_sb[:, 4:8, :], in_=wvd[:, 4:8, :])
        bk_sb = wpool.tile([128, 1], F32)
        nc.sync.dma_start(out=bk_sb[:], in_=bkd[:, :])
        bv_sb = wpool.tile([128, 1], F32)
        nc.sync.dma_start(out=bv_sb[:], in_=bvd[:, :])
        ident = const.tile([128, 128], F32R)
        nc.sync.dma_start(out=ident[:], in_=identd[:, :])

        wq_sb = wpool.tile([128, ET, CQ], BF16)
        wo_sb = wpool.tile([128, QT, E], F32R)
        bq_sb = wpool.tile([128, QT], F32)

        # ones row for the K=1 matmuls that broadcast 1/denom to 64 partitions
        ones_row = const.tile([1, 64], F32R)
        nc.sync.dma_start(out=ones_row[:], in_=onesd[0:1, :])

        # ---- persistent activations ----
        qT_sb = big.tile([128, QT, S], F32R)      # 32KB/partition
        kT_sb = big.tile([128, S], F32R)          # 8KB
        vT_sb = big.tile([128, S], F32R)          # 8KB
        vaug = big.tile([128, 2 * KT, 65], F32R)  # v natural + ones col
        # one strided DMA fills all 32 ones columns
        nc.sync.dma_start(out=vaug[:, :, 64:65], in_=onesd[:, 0:32])

        def dma_x_chunk(sc, name, pool=None, ways=8):
            xt = (pool or xpool).tile([128, ET, 512], BF16, tag="xt", name=name)
            lo = sc * 512
            step = ET // ways
            for w in range(ways):
                eb = w * step
                nc.sync.dma_start(out=xt[:, eb:eb + step, :],
                                  in_=xTd[:, eb:eb + step, lo:lo + 512])
            return xt

        def emit_kv(sc, xt):
            lo = sc * 512
            pkv = psA.tile([128, 1024], F32, tag="mm", name=f"pkv{sc}")
            for et in range(ET):
                nc.tensor.matmul(pkv[:, 0:512], wk_sb[:, et, :], xt[:, et, :],
                                 start=(et == 0), stop=(et == ET - 1))
            for et in range(ET):
                nc.tensor.matmul(pkv[:, 512:1024], wv_sb[:, et, :], xt[:, et, :],
                                 start=(et == 0), stop=(et == ET - 1))
            nc.vector.tensor_scalar(out=kT_sb[:, lo:lo + 512], in0=pkv[:, 0:512],
                                    scalar1=bk_sb[:, 0:1], scalar2=None, op0=ADD)
            nc.vector.tensor_scalar(out=vT_sb[:, lo:lo + 512], in0=pkv[:, 512:1024],
                                    scalar1=bv_sb[:, 0:1], scalar2=None, op0=ADD)
            # transpose vT chunk -> v natural tiles (ones col kept intact)
            for ktl in range(4):
                kt = sc * 4 + ktl
                ptr = psY.tile([128, 128], F32R, tag="y", name=f"ptr{kt}")
                nc.tensor.transpose(ptr[:], vT_sb[:, kt * 128:(kt + 1) * 128],
                                    ident[:])
                for g in range(2):
                    nc.vector.tensor_copy(
                        out=vaug[:, g * KT + kt, 0:64], in_=ptr[:, g * 64:(g + 1) * 64])

        # ================= q projection thunks for chunk qc =================
        def qproj_thunks(qc, xtq, tiles=None):
            """Lazily-allocating thunk list: per qT tile t, 8 matmuls + bias.
            Single-bank psY tiles so interleaving into the attention loop
            cannot wedge the 2-deep psA scores ring."""
            lo = qc * 512
            thunks = []
            for t in (range(QT) if tiles is None else tiles):
                cell = {}

                def mm(et, t=t, cell=cell, xtq=xtq, qc=qc):
                    if "pq" not in cell:
                        cell["pq"] = psY.tile([128, 512], F32, tag="y",
                                              name=f"pq{qc}_{t}")
                    nc.tensor.matmul(
                        cell["pq"][:], wq_sb[:, et, t * 128:(t + 1) * 128],
                        xtq[:, et, :], start=(et == 0), stop=(et == ET - 1))

                for et in range(ET):
                    thunks.append(lambda et=et, mm=mm: mm(et))

                def bias(t=t, cell=cell, lo=lo):
                    nc.vector.tensor_scalar(
                        out=qT_sb[:, t, lo:lo + 512], in0=cell["pq"][:],
                        scalar1=bq_sb[:, t:t + 1], scalar2=None, op0=ADD)
                thunks.append(bias)
            return thunks

        # ================= out projection thunks for chunk qc ===============
        def oproj_thunks(qc, avT, act_copy=False):
            lo = qc * 512
            thunks = []
            for et in range(ET):
                cell = {}

                def mm(t, et=et, cell=cell, avT=avT, qc=qc):
                    if "yp" not in cell:
                        cell["yp"] = psY.tile([128, 512], F32, tag="y",
                                              name=f"yp{qc}_{et}")
                    nc.tensor.matmul(
                        cell["yp"][:], wo_sb[:, t, et * 128:(et + 1) * 128],
                        avT[:, t, :], start=(t == 0), stop=(t == QT - 1))

                for t in range(QT):
                    thunks.append(lambda t=t, mm=mm: mm(t))

                def store(et=et, cell=cell, lo=lo, qc=qc):
                    ysb = ypool.tile([128, 512], F32, tag="ysb",
                                     name=f"ysb{qc}_{et}")
                    if act_copy:
                        # kernel tail: ACT is idle there, DVE is not
                        nc.scalar.copy(out=ysb[:], in_=cell["yp"][:])
                    else:
                        nc.vector.tensor_copy(out=ysb[:], in_=cell["yp"][:])
                    # two half-DMAs balance engine parallelism against the
                    # serial per-DMA HWDGE issue cost
                    for w in range(2):
                        o = lo + w * 256
                        nc.sync.dma_start(
                            out=yT[et * 128:(et + 1) * 128, o:o + 256],
                            in_=ysb[:, w * 256:(w + 1) * 256])
                thunks.append(store)
            return thunks

        # ================= phase A: kv projections + qproj(0,t0) ============
        # PE order kv0, kv1, qproj0[t0], kv2, kv3 tracks the DMA arrival order
        # of x chunks and wq; qT tiles 1-3 of chunk 0 are pumped into the
        # chunk-0 attention loop (pair t needs only tile t).
        xt1 = dma_x_chunk(1, "xt1", ways=4)
        for j in range(2):
            nc.sync.dma_start(out=wq_sb[:, 4 * j:4 * j + 4, :],
                              in_=wqd[:, 4 * j:4 * j + 4, :])
        nc.sync.dma_start(out=bq_sb[:], in_=bqd[:, :])
        xt2 = dma_x_chunk(2, "xt2", ways=4)
        xt3 = dma_x_chunk(3, "xt3", ways=4)
        emit_kv(0, xt0)
        emit_kv(1, xt1)
        for th in qproj_thunks(0, xt0, tiles=[0]):
            th()
        emit_kv(2, xt2)
        # kv chunk 3 is deferred into the first attention pair (its key tiles
        # aren't consumed until kt=12), letting ACT start ~6us earlier
        qproj0_rest = qproj_thunks(0, xt0, tiles=[1, 2, 3])

        # ================= attention: stream per q chunk =================
        def emit_av(avp, avpB, pe, kt):
            nc.tensor.matmul(avp[0:65, :], vaug[:, 0 * KT + kt, :],
                             pe[:, 0:512],
                             start=(kt == 0), stop=(kt == KT - 1))
            nc.tensor.matmul(avpB[0:65, :], vaug[:, 1 * KT + kt, :],
                             pe[:, 512:1024],
                             start=(kt == 0), stop=(kt == KT - 1))

        avT_tiles = {}
        pending_norm = [None]

        def emit_norm_pe_part(act_copy=False):
            """The deferred PE+DVE part of the previous pair's normalization:
            per head a K=1 matmul broadcasts the 1/denom row, then a DVE
            multiply writes the normalized avT block."""
            if pending_norm[0] is None:
                return
            avp, avpB, linvA, linvB, avT, t, qc = pending_norm[0]
            pending_norm[0] = None
            # DVE can read only one PSUM operand, so copy the broadcasts out
            for g, bank, linv in ((0, avp, linvA), (1, avpB, linvB)):
                ph = g * 64
                lrp = psY.tile([128, 512], F32, tag="y", name=f"lrp{qc}_{t}_{g}")
                nc.tensor.matmul(lrp[0:64, :], ones_row[:], linv[:],
                                 start=True, stop=True)
                lrep = npool.tile([64, 512], F32, tag="lrep",
                                  name=f"lrep{qc}_{t}_{g}")
                if act_copy:
                    # kernel tail: ACT is idle, keep DVE free for the MULs
                    nc.scalar.copy(out=lrep[:], in_=lrp[0:64, :])
                else:
                    nc.vector.tensor_copy(out=lrep[:], in_=lrp[0:64, :])
                nc.vector.tensor_tensor(
                    out=avT[ph:ph + 64, t, :], in0=bank[0:64, :], in1=lrep[:],
                    op=MUL)

        for qc in range(SC):
            lo = qc * 512
            avT = avpool.tile([128, QT, 512], F32R, tag="avT", name=f"avT{qc}")
            avT_tiles[qc] = avT

            # interleave queue: next chunk's q projection + previous chunk's
            # out projection, pumped one per kt into the attention loop.
            tasks = []

            def pump(k=1, tasks=tasks):
                for _ in range(k):
                    if tasks:
                        tasks.pop(0)()

            if qc == 0:
                tasks.extend(qproj0_rest)
                # wo streams behind the attention-critical transfers; first
                # use is the out-projection of chunk 0, pumped during chunk 1
                for t in range(QT):
                    nc.sync.dma_start(out=wo_sb[:, t, :], in_=wod[:, t, :])
            if qc < SC - 1:
                xtq = dma_x_chunk(qc + 1, f"xtq{qc + 1}")
                tasks.extend(qproj_thunks(qc + 1, xtq))
            if qc > 0:
                tasks.extend(oproj_thunks(qc - 1, avT_tiles[qc - 1]))

            for t in range(QT):          # head pair (t: group 0, t+4: group 1)
                avp = psAV.tile([128, 512], F32, tag="av", name=f"avp{qc}_{t}")
                avpB = psAV.tile([128, 512], F32, tag="av", name=f"avpB{qc}_{t}")
                pe_tiles = {}
                for kt in range(KT):
                    sp = psA.tile([128, 1024], F32, tag="mm",
                                  name=f"sp{qc}_{t}_{kt}")
                    nc.tensor.matmul(
                        sp[:, 0:512],
                        kT_sb[0:64, kt * 128:(kt + 1) * 128],
                        qT_sb[0:64, t, lo:lo + 512],
                        start=True, stop=True)
                    nc.tensor.matmul(
                        sp[:, 512:1024],
                        kT_sb[64:128, kt * 128:(kt + 1) * 128],
                        qT_sb[64:128, t, lo:lo + 512],
                        start=True, stop=True)
                    pe = ppool.tile([128, 1024], F32R, tag="pexp",
                                    name=f"pe{qc}_{t}_{kt}")
                    nc.scalar.activation(pe[:], sp[:], EXP, scale=float(SCALE))
                    pe_tiles[kt] = pe
                    if kt == 1:
                        # previous pair's deferred normalization: emitted
                        # after this pair's first scores so ACT never waits
                        emit_norm_pe_part()
                    if qc == 0 and t == 0 and kt == 5:
                        # deferred kv chunk 3, k half: ~1.9us of PE work that
                        # fits under the ACT exp backlog (kt>=12 needs kT sc3)
                        pk3 = psA.tile([128, 512], F32, tag="mm", name="pk3")
                        for et in range(ET):
                            nc.tensor.matmul(pk3[:], wk_sb[:, et, :],
                                             xt3[:, et, :],
                                             start=(et == 0), stop=(et == ET - 1))
                        nc.vector.tensor_scalar(
                            out=kT_sb[:, 1536:2048], in0=pk3[:],
                            scalar1=bk_sb[:, 0:1], scalar2=None, op0=ADD)
                    if qc == 0 and t == 0 and kt == 9:
                        # v half + transposes (AV kt>=12 needs vaug sc3)
                        pv3 = psA.tile([128, 512], F32, tag="mm", name="pv3")
                        for et in range(ET):
                            nc.tensor.matmul(pv3[:], wv_sb[:, et, :],
                                             xt3[:, et, :],
                                             start=(et == 0), stop=(et == ET - 1))
                        nc.vector.tensor_scalar(
                            out=vT_sb[:, 1536:2048], in0=pv3[:],
                            scalar1=bv_sb[:, 0:1], scalar2=None, op0=ADD)
                        for ktl in range(4):
                            kt3 = 12 + ktl
                            ptr = psY.tile([128, 128], F32R, tag="y",
                                           name=f"ptr{kt3}")
                            nc.tensor.transpose(
                                ptr[:], vT_sb[:, kt3 * 128:(kt3 + 1) * 128],
                                ident[:])
                            for g in range(2):
                                nc.vector.tensor_copy(
                                    out=vaug[:, g * KT + kt3, 0:64],
                                    in_=ptr[:, g * 64:(g + 1) * 64])
                    # software pipeline: AV trails scores by one kt
                    if kt > 0:
                        emit_av(avp, avpB, pe_tiles.pop(kt - 1), kt - 1)
                        pump()
                emit_av(avp, avpB, pe_tiles.pop(KT - 1), KT - 1)
                pump()

                # reciprocals of the softmax denominators (DVE, off PE path)
                linvA = npool.tile([1, 512], F32R, tag="linv",
                                   name=f"linvA{qc}_{t}")
                linvB = npool.tile([1, 512], F32R, tag="linv",
                                   name=f"linvB{qc}_{t}")
                nc.vector.reciprocal(linvA[:], avp[64:65, :])
                nc.vector.reciprocal(linvB[:], avpB[64:65, :])
                pending_norm[0] = (avp, avpB, linvA, linvB, avT, t, qc)

            while tasks:
                tasks.pop(0)()

        # last pair's normalization + final out projection
        emit_norm_pe_part(act_copy=True)
        for th in oproj_thunks(SC - 1, avT_tiles[SC - 1], act_copy=True):
            th()

    nc.compile()
    return nc


def _shard_inputs(x, Wq, bq, Wk, bk, Wv, bv, Wo, bo):
    """Build the 8 per-core input maps."""
    bf16 = ml_dtypes.bfloat16
    x = np.asarray(x, dtype=np.float32)

    def pmajor(w, dtype):
        # [n*128, m] -> [128, n, m] partition-major
        w = np.asarray(w, np.float32)
        n = w.shape[0] // 128
        return np.ascontiguousarray(
            w.reshape(n, 128, w.shape[1]).transpose(1, 0, 2).astype(dtype))

    in_maps = []
    for c in range(8):
        b, H = c // 2, c % 2
        heads = [8 * H + t for t in range(4)] + [8 * H + t + 4 for t in range(4)]
        # qT tile t holds (local head t -> partitions 0-63, head t+4 -> 64-127)
        order = []
        for t in range(4):
            order.extend(range(heads[t] * 64, heads[t] * 64 + 64))
            order.extend(range(heads[t + 4] * 64, heads[t + 4] * 64 + 64))
        order = np.asarray(order)
        wq_p = pmajor(np.asarray(Wq, np.float32)[:, order], bf16)
        bq_p = np.ascontiguousarray(
            np.asarray(bq, np.float32)[order].reshape(4, 128).T)
        wo_p = pmajor(np.asarray(Wo, np.float32)[order, :], np.float32)
        wk_s = pmajor(np.asarray(Wk, np.float32)[:, H * 128:(H + 1) * 128], bf16)
        wv_s = pmajor(np.asarray(Wv, np.float32)[:, H * 128:(H + 1) * 128], bf16)
        bk_s = np.ascontiguousarray(np.asarray(bk, np.float32)[H * 128:(H + 1) * 128]
                                    .reshape(128, 1))
        bv_s = np.ascontiguousarray(np.asarray(bv, np.float32)[H * 128:(H + 1) * 128]
                                    .reshape(128, 1))
        xT_b = pmajor(np.ascontiguousarray(x[b].T), bf16)
        in_maps.append({
            "xTd": xT_b, "wqd": wq_p, "wkd": wk_s, "wvd": wv_s, "wod": wo_p,
            "bqd": bq_p, "bkd": bk_s, "bvd": bv_s,
            "identd": np.eye(128, dtype=np.float32),
            "onesd": np.ones((128, 64), dtype=np.float32),
        })
    return in_maps


def _gather_output(results, inputs):
    """results: list of 8 per-core {name: array} dicts."""
    bo = np.asarray(inputs["bo"], dtype=np.float32)
    out = np.empty((B, S, E), dtype=np.float32)
    for b in range(B):
        yT = results[2 * b]["yT"] + results[2 * b + 1]["yT"]
        out[b] = yT.T + bo
    return out


def kernel(x, Wq, bq, Wk, bk, Wv, bv, Wo, bo, _trace=False):
    if "nc" not in _NC_CACHE:
        _NC_CACHE["nc"] = build_nc()
    nc = _NC_CACHE["nc"]
    in_maps = _shard_inputs(x, Wq, bq, Wk, bk, Wv, bv, Wo, bo)
    res = run_bass_kernel_spmd(nc, in_maps, list(range(8)), trace=_trace)
    out = _gather_output(res.results, {"bo": bo})
    if _trace:
        return out, res
    return out


# revision 57
# speedup vs baseline: 1.0118x; 1.0118x over previous
"""GQA self-attention kernel for Trainium2, sharded over 8 NeuronCores.

Problem: x[4, 2048, 1024], 16 heads / 4 KV groups / head_dim 64.
Sharding: batch (4) x head-half (2 KV groups each) = 8 cores.

Per-core dataflow (all "transposed world": features on partitions):
  xT[1024,2048] -> qT[512,2048], kT[128,2048], vT[128,2048]
      (PE matmuls; x and q/k/v weights stream in bf16, accumulate fp32)
  vT --PE transpose--> v_aug[seq,65] tiles (ones column for softmax sums)
  scores s[k,q] = kT_g^T(d,kpos) . qT_h(d,q)  (K=64, f32r; the two heads of a
      pair go to opposite PE-array row halves so hardware overlaps them)
  p = exp(s/8)  (ACT engine, 1024-wide tiles: both heads of a pair per kt)
  av[65,q] += v_aug^T p  (row 64 = softmax denominator)
  avT_norm = av[0:64] * recip(av[64])  (recips on DVE; one selector matmul
      broadcasts both heads' 1/denom rows; applied via DVE multiply)
  yT[e,q] = Wo_p^T . avT_norm  -> DRAM
Host: y[b] = (yT[2b] + yT[2b+1]).T + bo

Schedule: kv projections for all chunks first, then attention streams per
q-chunk. The next chunk's q-projection and the previous chunk's
out-projection matmuls are pumped one-per-kt into the ACT-bound attention
loop; each pair's normalization is deferred into the next pair's first
iterations so the scores pipeline never stalls.
"""

import sys
import numpy as np
import ml_dtypes

sys.path.insert(0, "/opt/trn_rl_repo")

from contextlib import ExitStack

import concourse.bass as bass
import concourse.bacc as bacc
import concourse.mybir as mybir
from concourse import tile
from concourse.bass_utils import run_bass_kernel_spmd

F32 = mybir.dt.float32
F32R = mybir.dt.float32r
BF16 = mybir.dt.bfloat16

B, S, E = 4, 2048, 1024
NUM_HEADS, NUM_GROUPS, D = 16, 4, 64
CQ = 512          # q cols per core (8 heads)
CK = 128          # kv cols per core (2 groups)
ET = E // 128     # 8 embed K-tiles
SC = S // 512     # 4 seq chunks of 512
KT = S // 128     # 16 key tiles of 128
QT = CQ // 128    # 4 qT partition tiles
SCALE = 1.0 / np.sqrt(np.float32(D))

_NC_CACHE = {}


def build_nc():
    nc = bacc.Bacc(None, target_bir_lowering=False)

    # partition-major layouts: [128, et, ...] so one DMA covers several
    # 128-row blocks; x and qkv weights are bf16 to halve the critical DMA
    xTd = nc.dram_tensor("xTd", [128, ET, S], BF16, kind="ExternalInput")
    wqd = nc.dram_tensor("wqd", [128, ET, CQ], BF16, kind="ExternalInput")
    wkd = nc.dram_tensor("wkd", [128, ET, CK], BF16, kind="ExternalInput")
    wvd = nc.dram_tensor("wvd", [128, ET, CK], BF16, kind="ExternalInput")
    wod = nc.dram_tensor("wod", [128, QT, E], F32R, kind="ExternalInput")
    bqd = nc.dram_tensor("bqd", [128, QT], F32, kind="ExternalInput")
    bkd = nc.dram_tensor("bkd", [128, 1], F32, kind="ExternalInput")
    bvd = nc.dram_tensor("bvd", [128, 1], F32, kind="ExternalInput")
    identd = nc.dram_tensor("identd", [128, 128], F32R, kind="ExternalInput")
    onesd = nc.dram_tensor("onesd", [128, 64], F32R, kind="ExternalInput")
    yT = nc.dram_tensor("yT", [E, S], F32, kind="ExternalOutput")

    ADD = mybir.AluOpType.add
    MUL = mybir.AluOpType.mult
    EXP = mybir.ActivationFunctionType.Exp

    with tile.TileContext(nc) as tc, ExitStack() as ctx, \
            nc.allow_low_precision(reason="attention math in f32r; x/qkv "
                                   "weight streams bf16 within tolerance"):
        const = ctx.enter_context(tc.tile_pool(name="const", bufs=1))
        wpool = ctx.enter_context(tc.tile_pool(name="wpool", bufs=1))
        big = ctx.enter_context(tc.tile_pool(name="big", bufs=1))
        x0pool = ctx.enter_context(tc.tile_pool(name="x0pool", bufs=1))
        xpool = ctx.enter_context(tc.tile_pool(name="xpool", bufs=3))
        ppool = ctx.enter_context(tc.tile_pool(name="ppool", bufs=4))
        avpool = ctx.enter_context(tc.tile_pool(name="avpool", bufs=3))
        ypool = ctx.enter_context(tc.tile_pool(name="ypool", bufs=4))
        npool = ctx.enter_context(tc.tile_pool(name="npool", bufs=8))
        # PSUM: psA 2x[128,1024] (4 banks) + psAV 2x[128,512] (2) + psY 2x (2)
        psA = ctx.enter_context(tc.tile_pool(name="psA", bufs=2, space="PSUM"))
        psAV = ctx.enter_context(tc.tile_pool(name="psAV", bufs=2, space="PSUM"))
        psY = ctx.enter_context(tc.tile_pool(name="psY", bufs=2, space="PSUM"))

        # ---- x chunk 0 + kv weights: the prologue-critical DMAs go first ----
        # 8-way split so 8 DMA engines fetch the first chunk concurrently
        xt0 = x0pool.tile([128, ET, 512], BF16, name="xt0")
        for et in range(ET):
            nc.sync.dma_start(out=xt0[:, et, :], in_=xTd[:, et, 0:512])
        wk_sb = wpool.tile([128, ET, CK], BF16)
        nc.sync.dma_start(out=wk_sb[:, 0:4, :], in_=wkd[:, 0:4, :])
        nc.sync.dma_start(out=wk_sb[:, 4:8, :], in_=wkd[:, 4:8, :])
        wv_sb = wpool.tile([128, ET, CK], BF16)
        nc.sync.dma_start(out=wv_sb[:, 0:4, :], in_=wvd[:, 0:4, :])
        nc.sync.dma_start(out=wv_sb[:, 4:8, :], in_=wvd[:, 4:8, :])
        bk_sb = wpool.tile([128, 1], F32)
        nc.sync.dma_start(out=bk_sb[:], in_=bkd[:, :])
        bv_sb = wpool.tile([128, 1], F32)
        nc.sync.dma_start(out=bv_sb[:], in_=bvd[:, :])
        ident = const.tile([128, 128], F32R)
        nc.sync.dma_start(out=ident[:], in_=identd[:, :])

        wq_sb = wpool.tile([128, ET, CQ], BF16)
        wo_sb = wpool.tile([128, QT, E], F32R)
        bq_sb = wpool.tile([128, QT], F32)

        # ones row for the K=1 matmuls that broadcast 1/denom to 64 partitions
        ones_row = const.tile([1, 64], F32R)
        nc.sync.dma_start(out=ones_row[:], in_=onesd[0:1, :])

        # ---- persistent activations ----
        qT_sb = big.tile([128, QT, S], F32R)      # 32KB/partition
        kT_sb = big.tile([128, S], F32R)          # 8KB
        vT_sb = big.tile([128, S], F32R)          # 8KB
        vaug = big.tile([128, 2 * KT, 65], F32R)  # v natural + ones col
        # one strided DMA fills all 32 ones columns
        nc.sync.dma_start(out=vaug[:, :, 64:65], in_=onesd[:, 0:32])

        def dma_x_chunk(sc, name, pool=None, ways=8):
            xt = (pool or xpool).tile([128, ET, 512], BF16, tag="xt", name=name)
            lo = sc * 512
            step = ET // ways
            for w in range(ways):
                eb = w * step
                nc.sync.dma_start(out=xt[:, eb:eb + step, :],
                                  in_=xTd[:, eb:eb + step, lo:lo + 512])
            return xt

        def emit_kv(sc, xt):
            lo = sc * 512
            pkv = psA.tile([128, 1024], F32, tag="mm", name=f"pkv{sc}")
            for et in range(ET):
                nc.tensor.matmul(pkv[:, 0:512], wk_sb[:, et, :], xt[:, et, :],
                                 start=(et == 0), stop=(et == ET - 1))
            for et in range(ET):
                nc.tensor.matmul(pkv[:, 512:1024], wv_sb[:, et, :], xt[:, et, :],
                                 start=(et == 0), stop=(et == ET - 1))
            nc.vector.tensor_scalar(out=kT_sb[:, lo:lo + 512], in0=pkv[:, 0:512],
                                    scalar1=bk_sb[:, 0:1], scalar2=None, op0=ADD)
            nc.vector.tensor_scalar(out=vT_sb[:, lo:lo + 512], in0=pkv[:, 512:1024],
                                    scalar1=bv_sb[:, 0:1], scalar2=None, op0=ADD)
            # transpose vT chunk -> v natural tiles (ones col kept intact)
            for ktl in range(4):
                kt = sc * 4 + ktl
                ptr = psY.tile([128, 128], F32R, tag="y", name=f"ptr{kt}")
                nc.tensor.transpose(ptr[:], vT_sb[:, kt * 128:(kt + 1) * 128],
                                    ident[:])
                for g in range(2):
                    nc.vector.tensor_copy(
                        out=vaug[:, g * KT + kt, 0:64], in_=ptr[:, g * 64:(g + 1) * 64])

        # ================= q projection thunks for chunk qc =================
        def qproj_thunks(qc, xtq, tiles=None):
            """Lazily-allocating thunk list: per qT tile t, 8 matmuls + bias.
            Single-bank psY tiles so interleaving into the attention loop
            cannot wedge the 2-deep psA scores ring."""
            lo = qc * 512
            thunks = []
            for t in (range(QT) if tiles is None else tiles):
                cell = {}

                def mm(et, t=t, cell=cell, xtq=xtq, qc=qc):
                    if "pq" not in cell:
                        cell["pq"] = psY.tile([128, 512], F32, tag="y",
                                              name=f"pq{qc}_{t}")
                    nc.tensor.matmul(
                        cell["pq"][:], wq_sb[:, et, t * 128:(t + 1) * 128],
                        xtq[:, et, :], start=(et == 0), stop=(et == ET - 1))

                for et in range(ET):
                    thunks.append(lambda et=et, mm=mm: mm(et))

                def bias(t=t, cell=cell, lo=lo):
                    nc.vector.tensor_scalar(
                        out=qT_sb[:, t, lo:lo + 512], in0=cell["pq"][:],
                        scalar1=bq_sb[:, t:t + 1], scalar2=None, op0=ADD)
                thunks.append(bias)
            return thunks

        # ================= out projection thunks for chunk qc ===============
        def oproj_thunks(qc, avT, act_copy=False):
            lo = qc * 512
            thunks = []
            for et in range(ET):
                cell = {}

                def mm(t, et=et, cell=cell, avT=avT, qc=qc):
                    if "yp" not in cell:
                        cell["yp"] = psY.tile([128, 512], F32, tag="y",
                                              name=f"yp{qc}_{et}")
                    nc.tensor.matmul(
                        cell["yp"][:], wo_sb[:, t, et * 128:(et + 1) * 128],
                        avT[:, t, :], start=(t == 0), stop=(t == QT - 1))

                for t in range(QT):
                    thunks.append(lambda t=t, mm=mm: mm(t))

                def store(et=et, cell=cell, lo=lo, qc=qc):
                    ysb = ypool.tile([128, 512], F32, tag="ysb",
                                     name=f"ysb{qc}_{et}")
                    if act_copy:
                        # kernel tail: ACT is idle there, DVE is not
                        nc.scalar.copy(out=ysb[:], in_=cell["yp"][:])
                    else:
                        nc.vector.tensor_copy(out=ysb[:], in_=cell["yp"][:])
                    # two half-DMAs balance engine parallelism against the
                    # serial per-DMA HWDGE issue cost
                    for w in range(2):
                        o = lo + w * 256
                        nc.sync.dma_start(
                            out=yT[et * 128:(et + 1) * 128, o:o + 256],
                            in_=ysb[:, w * 256:(w + 1) * 256])
                thunks.append(store)
            return thunks

        # ================= phase A: kv projections + qproj(0,t0) ============
        # PE order kv0, kv1, qproj0[t0], kv2, kv3 tracks the DMA arrival order
        # of x chunks and wq; qT tiles 1-3 of chunk 0 are pumped into the
        # chunk-0 attention loop (pair t needs only tile t).
        xt1 = dma_x_chunk(1, "xt1", ways=4)
        for j in range(2):
            nc.sync.dma_start(out=wq_sb[:, 4 * j:4 * j + 4, :],
                              in_=wqd[:, 4 * j:4 * j + 4, :])
        nc.sync.dma_start(out=bq_sb[:], in_=bqd[:, :])
        xt2 = dma_x_chunk(2, "xt2", ways=4)
        xt3 = dma_x_chunk(3, "xt3", ways=4)
        emit_kv(0, xt0)
        emit_kv(1, xt1)
        for th in qproj_thunks(0, xt0, tiles=[0]):
            th()
        emit_kv(2, xt2)
        # kv chunk 3 is deferred into the first attention pair (its key tiles
        # aren't consumed until kt=12), letting ACT start ~6us earlier
        qproj0_rest = qproj_thunks(0, xt0, tiles=[1, 2, 3])

        # ================= attention: stream per q chunk =================
        def emit_av(avp, avpB, pe, kt):
            nc.tensor.matmul(avp[0:65, :], vaug[:, 0 * KT + kt, :],
                             pe[:, 0:512],
                             start=(kt == 0), stop=(kt == KT - 1))
            nc.tensor.matmul(avpB[0:65, :], vaug[:, 1 * KT + kt, :],
                             pe[:, 512:1024],
                             start=(kt == 0), stop=(kt == KT - 1))

        avT_tiles = {}
        pending_norm = [None]

        def emit_norm_pe_part(act_copy=False):
            """The deferred PE+DVE part of the previous pair's normalization:
            per head a K=1 matmul broadcasts the 1/denom row, then a DVE
            multiply writes the normalized avT block."""
            if pending_norm[0] is None:
                return
            avp, avpB, linvA, linvB, avT, t, qc = pending_norm[0]
            pending_norm[0] = None
            # DVE can read only one PSUM operand, so copy the broadcasts out
            for g, bank, linv in ((0, avp, linvA), (1, avpB, linvB)):
                ph = g * 64
                lrp = psY.tile([128, 512], F32, tag="y", name=f"lrp{qc}_{t}_{g}")
                nc.tensor.matmul(lrp[0:64, :], ones_row[:], linv[:],
                                 start=True, stop=True)
                lrep = npool.tile([64, 512], F32, tag="lrep",
                                  name=f"lrep{qc}_{t}_{g}")
                if act_copy:
                    # kernel tail: ACT is idle, keep DVE free for the MULs
                    nc.scalar.copy(out=lrep[:], in_=lrp[0:64, :])
                else:
                    nc.vector.tensor_copy(out=lrep[:], in_=lrp[0:64, :])
                nc.vector.tensor_tensor(
                    out=avT[ph:ph + 64, t, :], in0=bank[0:64, :], in1=lrep[:],
                    op=MUL)

        for qc in range(SC):
            lo = qc * 512
            avT = avpool.tile([128, QT, 512], F32R, tag="avT", name=f"avT{qc}")
            avT_tiles[qc] = avT

            # interleave queue: next chunk's q projection + previous chunk's
            # out projection, pumped one per kt into the attention loop.
            tasks = []

            def pump(k=1, tasks=tasks):
                for _ in range(k):
                    if tasks:
                        tasks.pop(0)()

            if qc == 0:
                tasks.extend(qproj0_rest)
                # wo streams behind the attention-critical transfers; first
                # use is the out-projection of chunk 0, pumped during chunk 1
                for t in range(QT):
                    nc.sync.dma_start(out=wo_sb[:, t, :], in_=wod[:, t, :])
            if qc < SC - 1:
                xtq = dma_x_chunk(qc + 1, f"xtq{qc + 1}")
                tasks.extend(qproj_thunks(qc + 1, xtq))
            if qc > 0:
                tasks.extend(oproj_thunks(qc - 1, avT_tiles[qc - 1]))

            for t in range(QT):          # head pair (t: group 0, t+4: group 1)
                avp = psAV.tile([128, 512], F32, tag="av", name=f"avp{qc}_{t}")
                avpB = psAV.tile([128, 512], F32, tag="av", name=f"avpB{qc}_{t}")
                pe_tiles = {}
                for kt in range(KT):
                    sp = psA.tile([128, 1024], F32, tag="mm",
                                  name=f"sp{qc}_{t}_{kt}")
                    nc.tensor.matmul(
                        sp[:, 0:512],
                        kT_sb[0:64, kt * 128:(kt + 1) * 128],
                        qT_sb[0:64, t, lo:lo + 512],
                        start=True, stop=True)
                    nc.tensor.matmul(
                        sp[:, 512:1024],
                        kT_sb[64:128, kt * 128:(kt + 1) * 128],
                        qT_sb[64:128, t, lo:lo + 512],
                        start=True, stop=True)
                    pe = ppool.tile([128, 1024], F32R, tag="pexp",
                                    name=f"pe{qc}_{t}_{kt}")
                    nc.scalar.activation(pe[:], sp[:], EXP, scale=float(SCALE))
                    pe_tiles[kt] = pe
                    if kt == 1:
                        # previous pair's deferred normalization: emitted
                        # after this pair's first scores so ACT never waits
                        emit_norm_pe_part()
                    if qc == 0 and t == 0 and kt == 5:
                        # deferred kv chunk 3, k half: ~1.9us of PE work that
                        # fits under the ACT exp backlog (kt>=12 needs kT sc3)
                        pk3 = psA.tile([128, 512], F32, tag="mm", name="pk3")
                        for et in range(ET):
                            nc.tensor.matmul(pk3[:], wk_sb[:, et, :],
                                             xt3[:, et, :],
                                             start=(et == 0), stop=(et == ET - 1))
                        nc.vector.tensor_scalar(
                            out=kT_sb[:, 1536:2048], in0=pk3[:],
                            scalar1=bk_sb[:, 0:1], scalar2=None, op0=ADD)
                    if qc == 0 and t == 0 and kt == 9:
                        # v half + transposes (AV kt>=12 needs vaug sc3)
                        pv3 = psA.tile([128, 512], F32, tag="mm", name="pv3")
                        for et in range(ET):
                            nc.tensor.matmul(pv3[:], wv_sb[:, et, :],
                                             xt3[:, et, :],
                                             start=(et == 0), stop=(et == ET - 1))
                        nc.vector.tensor_scalar(
                            out=vT_sb[:, 1536:2048], in0=pv3[:],
                            scalar1=bv_sb[:, 0:1], scalar2=None, op0=ADD)
                        for ktl in range(4):
                            kt3 = 12 + ktl
                            ptr = psY.tile([128, 128], F32R, tag="y",
                                           name=f"ptr{kt3}")
                            nc.tensor.transpose(
                                ptr[:], vT_sb[:, kt3 * 128:(kt3 + 1) * 128],
                                ident[:])
                            for g in range(2):
                                nc.vector.tensor_copy(
                                    out=vaug[:, g * KT + kt3, 0:64],
                                    in_=ptr[:, g * 64:(g + 1) * 64])
                    # software pipeline: AV trails scores by one kt
                    if kt > 0:
                        emit_av(avp, avpB, pe_tiles.pop(kt - 1), kt - 1)
                        pump()
                emit_av(avp, avpB, pe_tiles.pop(KT - 1), KT - 1)
                pump()

                # reciprocals of the softmax denominators (DVE, off PE path)
                linvA = npool.tile([1, 512], F32R, tag="linv",
                                   name=f"linvA{qc}_{t}")
                linvB = npool.tile([1, 512], F32R, tag="linv",
                                   name=f"linvB{qc}_{t}")
                nc.vector.reciprocal(linvA[:], avp[64:65, :])
                nc.vector.reciprocal(linvB[:], avpB[64:65, :])
                pending_norm[0] = (avp, avpB, linvA, linvB, avT, t, qc)

            while tasks:
                tasks.pop(0)()

        # last pair's normalization + final out projection
        emit_norm_pe_part(act_copy=True)
        for th in oproj_thunks(SC - 1, avT_tiles[SC - 1], act_copy=True):
            th()

    nc.compile()
    return nc


def _shard_inputs(x, Wq, bq, Wk, bk, Wv, bv, Wo, bo):
    """Build the 8 per-core input maps."""
    bf16 = ml_dtypes.bfloat16
    x = np.asarray(x, dtype=np.float32)

    def pmajor(w, dtype):
        # [n*128, m] -> [128, n, m] partition-major
        w = np.asarray(w, np.float32)
        n = w.shape[0] // 128
        return np.ascontiguousarray(
            w.reshape(n, 128, w.shape[1]).transpose(1, 0, 2).astype(dtype))

    in_maps = []
    for c in range(8):
        b, H = c // 2, c % 2
        heads = [8 * H + t for t in range(4)] + [8 * H + t + 4 for t in range(4)]
        # qT tile t holds (local head t -> partitions 0-63, head t+4 -> 64-127)
        order = []
        for t in range(4):
            order.extend(range(heads[t] * 64, heads[t] * 64 + 64))
            order.extend(range(heads[t + 4] * 64, heads[t + 4] * 64 + 64))
        order = np.asarray(order)
        wq_p = pmajor(np.asarray(Wq, np.float32)[:, order], bf16)
        bq_p = np.ascontiguousarray(
            np.asarray(bq, np.float32)[order].reshape(4, 128).T)
        wo_p = pmajor(np.asarray(Wo, np.float32)[order, :], np.float32)
        wk_s = pmajor(np.asarray(Wk, np.float32)[:, H * 128:(H + 1) * 128], bf16)
        wv_s = pmajor(np.asarray(Wv, np.float32)[:, H * 128:(H + 1) * 128], bf16)
        bk_s = np.ascontiguousarray(np.asarray(bk, np.float32)[H * 128:(H + 1) * 128]
                                    .reshape(128, 1))
        bv_s = np.ascontiguousarray(np.asarray(bv, np.float32)[H * 128:(H + 1) * 128]
                                    .reshape(128, 1))
        xT_b = pmajor(np.ascontiguousarray(x[b].T), bf16)
        in_maps.append({
            "xTd": xT_b, "wqd": wq_p, "wkd": wk_s, "wvd": wv_s, "wod": wo_p,
            "bqd": bq_p, "bkd": bk_s, "bvd": bv_s,
            "identd": np.eye(128, dtype=np.float32),
            "onesd": np.ones((128, 64), dtype=np.float32),
        })
    return in_maps


def _gather_output(results, inputs):
    """results: list of 8 per-core {name: array} dicts."""
    bo = np.asarray(inputs["bo"], dtype=np.float32)
    out = np.empty((B, S, E), dtype=np.float32)
    for b in range(B):
        yT = results[2 * b]["yT"] + results[2 * b + 1]["yT"]
        out[b] = yT.T + bo
    return out


def kernel(x, Wq, bq, Wk, bk, Wv, bv, Wo, bo, _trace=False):
    if "nc" not in _NC_CACHE:
        _NC_CACHE["nc"] = build_nc()
    nc = _NC_CACHE["nc"]
    in_maps = _shard_inputs(x, Wq, bq, Wk, bk, Wv, bv, Wo, bo)
    res = run_bass_kernel_spmd(nc, in_maps, list(range(8)), trace=_trace)
    out = _gather_output(res.results, {"bo": bo})
    if _trace:
        return out, res
    return out
